# revision 10
# baseline (speedup 1.0000x reference)
"""Trainium2 Bass kernel for a local-attention transformer block.

Strategy: data-parallel over tokens. B*S = 2*4096 = 8192 tokens are split
into 8 shards of 1024 tokens (4 shards per batch element, so no shard
crosses a batch boundary). Each core gets its shard plus a 128-token halo
of preceding tokens (zeros at batch start), computes QKV for the halo'd
range, runs banded sliding-window attention (each 128-query block attends
to exactly two 128-key blocks), out-projection, LN1, FFN (exact gelu),
LN2 — entirely locally, no collectives. Matmuls run in bf16 with fp32
accumulation; softmax/layernorm/residual paths stay fp32.
"""

import numpy as np
import ml_dtypes

# ---- problem constants (hardcoded per contract) ----
B, S, D = 2, 4096, 768
NH, HD = 12, 64
DFF = 4 * D            # 3072
DQK = 2 * D            # 1536 (Q and K channels)
WIN = 128              # sliding window
EPS = 1e-5
T = 128                # tile (partition) size
NB = 8                 # own 128-token blocks per core
NBH = NB + 1           # with one halo block
NTOK = NB * T          # 1024 own tokens per core
NTOKH = NBH * T        # 1152 with halo
ND = D // T            # 6
NC2 = DQK // T         # 12
NF = DFF // T          # 24
N_CORES = 8
NEG = -1e30

_CACHE = {}


def _build_nc(act="gelu"):
    import concourse.bacc as bacc
    import concourse.mybir as mybir
    from concourse import tile
    from concourse.masks import make_identity
    from contextlib import ExitStack

    f32 = mybir.dt.float32
    bf16 = mybir.dt.bfloat16
    AF = mybir.ActivationFunctionType
    ALU = mybir.AluOpType
    AX = mybir.AxisListType

    nc = bacc.Bacc("TRN2", target_bir_lowering=False, debug=False,
                   num_devices=N_CORES)

    # ---- DRAM I/O ----
    xh_d = nc.dram_tensor("xh", [NTOKH, D], f32, kind="ExternalInput").ap()
    mf_d = nc.dram_tensor("mask_first", [T, 2 * T], f32, kind="ExternalInput").ap()
    mr_d = nc.dram_tensor("mask_rest", [T, 2 * T], f32, kind="ExternalInput").ap()
    wqk_d = nc.dram_tensor("wqkT", [D, DQK], bf16, kind="ExternalInput").ap()
    wv_d = nc.dram_tensor("wvT", [D, D], bf16, kind="ExternalInput").ap()
    wo_d = nc.dram_tensor("woT", [D, D], bf16, kind="ExternalInput").ap()
    w1_d = nc.dram_tensor("w1T", [D, DFF], bf16, kind="ExternalInput").ap()
    w2_d = nc.dram_tensor("w2T", [DFF, D], bf16, kind="ExternalInput").ap()
    qkb_d = nc.dram_tensor("qkb", [T, NC2], f32, kind="ExternalInput").ap()
    b1c_d = nc.dram_tensor("b1c", [T, NF], f32, kind="ExternalInput").ap()
    # row vectors replicated to 128 partitions on host
    rep_names = ["vb", "ob", "b2", "g1", "bb1", "g2", "bb2"]
    reps_d = {n: nc.dram_tensor(f"rep_{n}", [T, D], f32, kind="ExternalInput").ap()
              for n in rep_names}
    out_d = nc.dram_tensor("out", [NTOK, D], f32, kind="ExternalOutput").ap()

    with tile.TileContext(nc) as tc, ExitStack() as ctx:
        persist = ctx.enter_context(tc.tile_pool(name="persist", bufs=1))
        ident = persist.tile([T, T], bf16, tag="ident")
        make_identity(nc, ident[:])
        yT_all = persist.tile([T, NB, ND, T], bf16, tag="yT")
        mf_sb = persist.tile([T, 2 * T], f32, tag="mf")
        nc.sync.dma_start(mf_sb[:], mf_d[:])
        mr_sb = persist.tile([T, 2 * T], f32, tag="mr")
        nc.sync.dma_start(mr_sb[:], mr_d[:])
        qkb_sb = persist.tile([T, NC2], f32, tag="qkb")
        nc.sync.dma_start(qkb_sb[:], qkb_d[:])
        b1c_sb = persist.tile([T, NF], f32, tag="b1c")
        nc.sync.dma_start(b1c_sb[:], b1c_d[:])
        eps_sb = persist.tile([T, 1], f32, tag="eps")
        nc.gpsimd.memset(eps_sb[:], EPS)
        rep_sb = {}
        for n in rep_names:
            rep_sb[n] = persist.tile([T, D], f32, tag=f"rep_{n}",
                                     name=f"rep_{n}_sb")
            nc.sync.dma_start(rep_sb[n][:], reps_d[n][:])

        def emit_ln(pool, ps_pool, xin, g_rep, b_rep, out_tag):
            ns = pool.tile([T, 1], f32, tag="ln_ns")
            nc.vector.tensor_reduce(ns[:], xin[:], axis=AX.X, op=ALU.add,
                                    negate=True)
            nm = pool.tile([T, 1], f32, tag="ln_nm")
            nc.scalar.mul(nm[:], ns[:], 1.0 / D)
            xc = pool.tile([T, D], f32, tag="ln_xc")
            nc.vector.tensor_scalar_add(xc[:], xin[:], nm[:])
            sq = pool.tile([T, D], f32, tag="ln_sq")
            vs = pool.tile([T, 1], f32, tag="ln_vs")
            nc.vector.scalar_tensor_tensor(sq[:], xc[:], 1.0, xc[:],
                                           op0=ALU.mult, op1=ALU.mult,
                                           accum_out=vs[:])
            std = pool.tile([T, 1], f32, tag="ln_std")
            nc.scalar.activation(std[:], vs[:], AF.Sqrt, bias=eps_sb[:],
                                 scale=1.0 / D)
            rstd = pool.tile([T, 1], f32, tag="ln_rstd")
            nc.vector.reciprocal(rstd[:], std[:])
            xg = pool.tile([T, D], f32, tag="ln_xg")
            nc.vector.scalar_tensor_tensor(xg[:], xc[:], rstd[:], g_rep[:],
                                           op0=ALU.mult, op1=ALU.mult)
            out = pool.tile([T, D], f32, tag=out_tag)
            nc.vector.tensor_tensor(out[:], xg[:], b_rep[:], op=ALU.add)
            return out

        # ================= phase A: QKV generation + attention =============
        with tc.tile_pool(name="wA", bufs=1) as wA, \
             tc.tile_pool(name="kv", bufs=1) as kv:
            wqk_sb = wA.tile([T, ND, DQK], bf16, tag="wqk")
            nc.sync.dma_start(wqk_sb[:], wqk_d.rearrange("(j p) n -> p j n", p=T))
            wv_sb = wA.tile([T, ND, D], bf16, tag="wv")
            nc.sync.dma_start(wv_sb[:], wv_d.rearrange("(j p) n -> p j n", p=T))
            qkT_sb = kv.tile([T, NC2, NTOKH], bf16, tag="qkT")
            v_sb = kv.tile([T, NBH, D], bf16, tag="v")

            with tc.tile_pool(name="workA", bufs=2) as workA, \
                 tc.tile_pool(name="psA", bufs=2, space="PSUM") as psA:
                for i in range(NBH):
                    x_sb = workA.tile([T, D], f32, tag="x")
                    nc.sync.dma_start(x_sb[:], xh_d[i * T:(i + 1) * T, :])
                    xb = workA.tile([T, D], bf16, tag="xb")
                    nc.vector.tensor_copy(xb[:], x_sb[:])
                    xT = workA.tile([T, ND, T], bf16, tag="xT")
                    for j in range(ND):
                        ptr = psA.tile([T, T], bf16, tag="tr")
                        nc.tensor.transpose(ptr[:], xb[:, j * T:(j + 1) * T],
                                            ident[:])
                        nc.vector.tensor_copy(xT[:, j, :], ptr[:])
                    # Q,K in [channel, token] layout
                    for ci in range(NC2):
                        pqk = psA.tile([T, T], f32, tag="qk")
                        for j in range(ND):
                            nc.tensor.matmul(pqk[:],
                                             wqk_sb[:, j, ci * T:(ci + 1) * T],
                                             xT[:, j, :],
                                             start=(j == 0), stop=(j == ND - 1))
                        nc.scalar.activation(qkT_sb[:, ci, i * T:(i + 1) * T],
                                             pqk[:], AF.Identity,
                                             bias=qkb_sb[:, ci:ci + 1])
                    # V in [token, channel] layout
                    for nh in range(2):
                        sl = slice(nh * 384, (nh + 1) * 384)
                        pv = psA.tile([T, 384], f32, tag="v")
                        for j in range(ND):
                            nc.tensor.matmul(pv[:], xT[:, j, :],
                                             wv_sb[:, j, sl],
                                             start=(j == 0), stop=(j == ND - 1))
                        nc.vector.tensor_tensor(v_sb[:, i, sl], pv[:],
                                                rep_sb["vb"][:, sl], op=ALU.add)

            # ---- banded attention: query block t sees key blocks t, t+1 ----
            with tc.tile_pool(name="attnA", bufs=3) as attnA, \
                 tc.tile_pool(name="psS", bufs=2, space="PSUM") as psS:
                for t in range(NB):
                    msk = mf_sb if t == 0 else mr_sb
                    for h in range(NH):
                        ci = h // 2
                        po = (h % 2) * HD
                        ps_s = psS.tile([T, 2 * T], f32, tag="s")
                        qT = qkT_sb[po:po + HD, ci, (t + 1) * T:(t + 2) * T]
                        kT = qkT_sb[po:po + HD, ND + ci, t * T:(t + 2) * T]
                        nc.tensor.matmul(ps_s[:], qT, kT, start=True, stop=True)
                        S_sb = attnA.tile([T, 2 * T], f32, tag="S")
                        nc.vector.tensor_tensor(S_sb[:], ps_s[:], msk[:],
                                                op=ALU.add)
                        P = attnA.tile([T, 2 * T], bf16, tag="P")
                        den = attnA.tile([T, 1], f32, tag="den")
                        nc.scalar.activation(P[:], S_sb[:], AF.Exp,
                                             scale=0.125, accum_out=den[:])
                        rec = attnA.tile([T, 1], f32, tag="rec")
                        nc.vector.reciprocal(rec[:], den[:])
                        Pn = attnA.tile([T, 2 * T], bf16, tag="Pn")
                        nc.vector.tensor_scalar_mul(Pn[:], P[:], rec[:])
                        ps_pt = psS.tile([T, 2 * T], bf16, tag="pt")
                        nc.tensor.transpose(ps_pt[:, 0:T], Pn[:, 0:T], ident[:])
                        nc.tensor.transpose(ps_pt[:, T:2 * T], Pn[:, T:2 * T],
                                            ident[:])
                        PT = attnA.tile([T, 2 * T], bf16, tag="PT")
                        nc.scalar.copy(PT[:], ps_pt[:])
                        ps_y = psS.tile([HD, T], f32, tag="y")
                        nc.tensor.matmul(ps_y[:],
                                         v_sb[:, t, h * HD:(h + 1) * HD],
                                         PT[:, 0:T], start=True, stop=False)
                        nc.tensor.matmul(ps_y[:],
                                         v_sb[:, t + 1, h * HD:(h + 1) * HD],
                                         PT[:, T:2 * T], start=False, stop=True)
                        nc.vector.tensor_copy(
                            yT_all[po:po + HD, t, ci, :], ps_y[:])

        # ============ phase B: out-proj + LN1 + FFN + LN2 ==================
        with tc.tile_pool(name="wB", bufs=1) as wB:
            wo_sb = wB.tile([T, ND, D], bf16, tag="wo")
            nc.sync.dma_start(wo_sb[:], wo_d.rearrange("(j p) n -> p j n", p=T))
            w1_sb = wB.tile([T, ND, DFF], bf16, tag="w1")
            nc.sync.dma_start(w1_sb[:], w1_d.rearrange("(j p) n -> p j n", p=T))
            w2_sb = wB.tile([T, NF, D], bf16, tag="w2")
            nc.sync.dma_start(w2_sb[:], w2_d.rearrange("(j p) n -> p j n", p=T))

            with tc.tile_pool(name="workB", bufs=2) as workB, \
                 tc.tile_pool(name="psB", bufs=2, space="PSUM") as psB:
                for t in range(NB):
                    xo = workB.tile([T, D], f32, tag="xo")
                    nc.sync.dma_start(xo[:], xh_d[(t + 1) * T:(t + 2) * T, :])
                    x1pre = workB.tile([T, D], f32, tag="x1pre")
                    for nh in range(2):
                        sl = slice(nh * 384, (nh + 1) * 384)
                        pz = psB.tile([T, 384], f32, tag="mm")
                        for j in range(ND):
                            nc.tensor.matmul(pz[:], yT_all[:, t, j, :],
                                             wo_sb[:, j, sl],
                                             start=(j == 0), stop=(j == ND - 1))
                        nc.vector.tensor_tensor(x1pre[:, sl], pz[:], xo[:, sl],
                                                op=ALU.add)
                        nc.vector.tensor_tensor(x1pre[:, sl], x1pre[:, sl],
                                                rep_sb["ob"][:, sl], op=ALU.add)
                    x1 = emit_ln(workB, psB, x1pre, rep_sb["g1"], rep_sb["bb1"],
                                 "x1")
                    x1b = workB.tile([T, D], bf16, tag="x1b")
                    nc.vector.tensor_copy(x1b[:], x1[:])
                    x1T = workB.tile([T, ND, T], bf16, tag="x1T")
                    for j in range(ND):
                        ptr = psB.tile([T, T], bf16, tag="tr")
                        nc.tensor.transpose(ptr[:], x1b[:, j * T:(j + 1) * T],
                                            ident[:])
                        nc.vector.tensor_copy(x1T[:, j, :], ptr[:])
                    h_sb = workB.tile([T, NF, T], bf16, tag="h")
                    for fi in range(NF):
                        ph = psB.tile([T, T], f32, tag="h1")
                        for j in range(ND):
                            nc.tensor.matmul(ph[:],
                                             w1_sb[:, j, fi * T:(fi + 1) * T],
                                             x1T[:, j, :],
                                             start=(j == 0), stop=(j == ND - 1))
                        act_fn = AF.Gelu if act == "gelu" else AF.Identity
                        nc.scalar.activation(h_sb[:, fi, :], ph[:], act_fn,
                                             bias=b1c_sb[:, fi:fi + 1])
                    x2pre = workB.tile([T, D], f32, tag="x2pre")
                    for nh in range(2):
                        sl = slice(nh * 384, (nh + 1) * 384)
                        pz2 = psB.tile([T, 384], f32, tag="mm")
                        for fi in range(NF):
                            nc.tensor.matmul(pz2[:], h_sb[:, fi, :],
                                             w2_sb[:, fi, sl],
                                             start=(fi == 0),
                                             stop=(fi == NF - 1))
                        nc.vector.tensor_tensor(x2pre[:, sl], pz2[:], x1[:, sl],
                                                op=ALU.add)
                        nc.vector.tensor_tensor(x2pre[:, sl], x2pre[:, sl],
                                                rep_sb["b2"][:, sl], op=ALU.add)
                    out_sb = emit_ln(workB, psB, x2pre, rep_sb["g2"],
                                     rep_sb["bb2"], "outb")
                    nc.sync.dma_start(out_d[t * T:(t + 1) * T, :], out_sb[:])

    nc.compile()
    return nc


def _get_nc(act="gelu"):
    if act not in _CACHE:
        _CACHE[act] = _build_nc(act)
    return _CACHE[act]


def make_in_maps(x, in_proj_w, in_proj_b, out_w, out_b, ff_w1, ff_b1,
                 ff_w2, ff_b2, n1_g, n1_b, n2_g, n2_b):
    bf = ml_dtypes.bfloat16
    f32 = np.float32
    x = np.asarray(x, f32).reshape(B, S, D)

    shared = {
        "wqkT": np.ascontiguousarray(np.asarray(in_proj_w, f32)[:DQK].T).astype(bf),
        "wvT": np.ascontiguousarray(np.asarray(in_proj_w, f32)[DQK:].T).astype(bf),
        "woT": np.ascontiguousarray(np.asarray(out_w, f32).T).astype(bf),
        "w1T": np.ascontiguousarray(np.asarray(ff_w1, f32).T).astype(bf),
        "w2T": np.ascontiguousarray(np.asarray(ff_w2, f32).T).astype(bf),
        "qkb": np.ascontiguousarray(
            np.asarray(in_proj_b, f32)[:DQK].reshape(NC2, T).T),
        "b1c": np.ascontiguousarray(np.asarray(ff_b1, f32).reshape(NF, T).T),
    }
    for name, vec in [("vb", np.asarray(in_proj_b, f32)[DQK:]),
                      ("ob", out_b), ("b2", ff_b2), ("g1", n1_g),
                      ("bb1", n1_b), ("g2", n2_g), ("bb2", n2_b)]:
        shared[f"rep_{name}"] = np.ascontiguousarray(
            np.broadcast_to(np.asarray(vec, f32)[None, :], (T, D)))

    q = np.arange(T, dtype=np.int64)[:, None]
    k = np.arange(T, dtype=np.int64)[None, :]
    M0 = np.where(k > q, 0.0, NEG).astype(f32)
    M1 = np.where(k <= q, 0.0, NEG).astype(f32)
    mask_rest = np.ascontiguousarray(np.concatenate([M0, M1], axis=1))
    mask_first_bs = np.ascontiguousarray(
        np.concatenate([np.full((T, T), NEG, f32), M1], axis=1))

    in_maps = []
    for c in range(N_CORES):
        b, i0 = divmod(c * NTOK, S)
        halo = (np.zeros((T, D), f32) if i0 == 0
                else x[b, i0 - T:i0])
        xh = np.ascontiguousarray(
            np.concatenate([halo, x[b, i0:i0 + NTOK]], axis=0))
        m = dict(shared)
        m["xh"] = xh
        m["mask_first"] = mask_first_bs if i0 == 0 else mask_rest
        m["mask_rest"] = mask_rest
        in_maps.append(m)
    return in_maps


def kernel(**inputs):
    from concourse.bass_utils import run_bass_kernel_spmd
    nc = _get_nc()
    in_maps = make_in_maps(**inputs)
    res = run_bass_kernel_spmd(nc, in_maps, core_ids=list(range(N_CORES)))
    outs = [res.results[c]["out"] for c in range(N_CORES)]
    return np.concatenate(outs, axis=0).reshape(B, S, D).astype(np.float32)
